# revision 33
# baseline (speedup 1.0000x reference)
"""AttentionBlock (GroupNorm(32) + 1-head self-attention + proj + residual) on 8 trn2 cores.

Data-parallel over batch: each of the 8 NeuronCores processes 2 of the 16 images.
All large matmuls run in fp8-e4m3 with perf_mode=DoubleRow (2 MACs/cell/cycle:
each matmul contracts K=256 via paired k-tiles in a 3D [128, 2, free] AP).

Weight merging (NUM_HEADS=1 makes adjacent linear maps composable host-side):
  - scores = xn^T (scale*Wq^T Wk) xn: precompute M, then t = M@xn (1 unit)
    + gram t(x)xn (2 units) replaces q,k,scores (4 units).
  - out = attn @ ((Wp Wv) xn): precompute W2, then v' = W2@xn (1) + attn@v'
    (2) replaces v,attn@v,proj (4). The attention output needs NO fp8
    roundtrip: PSUM -> (1/l mul) -> (+x residual) -> bf16 DMA directly.
  Net: 96 DoubleRow matmuls/image instead of 128 (25% PE cut), and fewer fp8
  quantization stages (better accuracy than the unmerged fp8 kernel).

Quantization scheme (tolerance is 2e-2; this lands ~1e-2):
  - M quantized at SM=512 (sigma_M ~ 0.002), W2 at SW2=32.
  - t stored as St*(M@xn), St=16; scores psum = St*score and the exp
    activation applies scale=1/St, bias=-1.5. The -1.5 shift keeps
    exp <= ~60 (TRN e4m3 overflows to Inf above 240) and cancels in softmax.
  - GroupNorm stats off the PE: per-channel mean/var via DVE bn_stats/bn_aggr,
    combined across the 16 channels/group by a tiny f32r indicator matmul.
  - Softmax denominator off the PE: a 3-add DVE tree collapses the 8 expT
    tiles, then one small f32r ones-matmul reduces partitions + broadcasts.

Layout strategy (per image, c=512 channels, n=1024 positions):
  - x bf16 [128, CT, N] (bf16 in/out DMA halves HBM traffic); xn/t fp8
    [128, CT, N] (channel, position); v'T fp8 [128, NT, C] (position,
    channel); expT fp8 [128, NT, N]. Pair-tiles adjacent in the free dim
    enable the [:, 2u:2u+2, ...] DoubleRow slices for both operands.
  - softmax over j: no max subtraction (scores ~N(0,1) and the -1.5 exp
    shift bounds the range).
  - rstd via Newton rsqrt on DVE; 1/l via reciprocal_approx_fast; ScalarE only
    needs one ACT table set (exp/copy/identity).
  - DMAs batched; per-phase emission order software-pipelines the two images.
"""

import ml_dtypes
import numpy as np

import concourse.bacc as bacc
import concourse.tile as tile
import concourse.mybir as mybir
from concourse.bass_utils import run_bass_kernel_spmd

F32 = mybir.dt.float32
F32R = mybir.dt.float32r
BF16 = mybir.dt.bfloat16
FP8 = mybir.dt.float8e4
I32 = mybir.dt.int32
AF = mybir.ActivationFunctionType
ALU = mybir.AluOpType
AX = mybir.AxisListType
DR = mybir.MatmulPerfMode.DoubleRow
FP8NP = ml_dtypes.float8_e4m3

B, C, H, W = 16, 512, 32, 32
N = H * W                 # 1024 positions
NCORES = 8
BPC = B // NCORES         # 2 images per core
G = 32                    # groupnorm groups
GS = C // G               # 16 channels per group
CT = C // 128             # 4 channel tiles
NT = N // 128             # 8 position tiles
NH = N // 512             # 2 free-dim halves
EPS = 1e-5
SCALE = float(C) ** -0.5  # single head, head_dim = C
MAGIC = 0x5F3759DF        # Newton-rsqrt seed constant

SM = 512.0                # M = scale*Wq^T@Wk quantization scale
SW2 = 32.0                # W2 = Wp@Wv quantization scale
ST = 16.0                 # t = M@xn storage prescale
SWC = 32.0                # bias-correction vector scale (nonzero-bias path)
EXPB = -1.5               # exp shift (cancels in softmax; keeps exp < 240)
TS = ST / SM              # t copy-out scale
VS = 1.0 / SW2            # v' copy-out scale

_cache: dict = {}

ATTN_DT = "fp8"


def _build(repeat: int = 1, zero_qk_bias: bool = True, loop_iters: int = 0,
           attn_dtype=None):
    nc = bacc.Bacc("TRN2", target_bir_lowering=False, num_devices=NCORES)

    x_d = nc.dram_tensor("x", [BPC, C, N], BF16, kind="ExternalInput")
    mt_d = nc.dram_tensor("mt", [C, C], FP8, kind="ExternalInput")     # M.T * SM
    w2t_d = nc.dram_tensor("w2t", [C, C], FP8, kind="ExternalInput")   # W2.T * SW2
    ind_d = nc.dram_tensor("ind", [C, G], F32R, kind="ExternalInput")  # 1/16 iff c//16==g
    bind_d = nc.dram_tensor("bind", [G, C], F32R, kind="ExternalInput")  # 0/1 indicator.T
    # consts: [gnsc | gnbi], each (128, CT)
    consts_d = nc.dram_tensor("consts", [128, 2 * CT], F32, kind="ExternalInput")
    if not zero_qk_bias:
        wc_d = nc.dram_tensor("wc", [C, 16], FP8, kind="ExternalInput")  # SWC*scale*Wk^T@bq (col 0)
    out_d = nc.dram_tensor("out", [BPC, C, N], BF16, kind="ExternalOutput")

    with tile.TileContext(nc) as tc:
        with (
            tc.tile_pool(name="wpool", bufs=1) as wp_,
            tc.tile_pool(name="xpool", bufs=2) as xpool,
            tc.tile_pool(name="xnpool", bufs=2) as xnpool,
            tc.tile_pool(name="espool", bufs=1) as espool,
            tc.tile_pool(name="qpool", bufs=2) as qpool,
            tc.tile_pool(name="vpool", bufs=2) as vpool,
            tc.tile_pool(name="epool", bufs=2) as epool,
            tc.tile_pool(name="fpool", bufs=1) as fpool,
            tc.tile_pool(name="tpool", bufs=2) as tpool,
            tc.tile_pool(name="rpool", bufs=2) as rpool,
            tc.tile_pool(name="spool", bufs=2) as spool,
            tc.tile_pool(name="psA", bufs=3, space="PSUM") as psA,
            tc.tile_pool(name="psB", bufs=2, space="PSUM") as psB,
        ):
            # ---- persistent constants / weights (batched single DMAs) ----
            mt_all = wp_.tile([128, CT, C], FP8, tag="mt", name="mt")
            w2t_all = wp_.tile([128, CT, C], FP8, tag="w2t", name="w2t")
            ind_all = wp_.tile([128, CT, G], F32R, tag="ind", name="ind")
            bind_all = wp_.tile([G, CT, 128], F32R, tag="bind", name="bind")
            ones_sb = wp_.tile([128, 128], F32R, tag="ones", name="ones")
            consts_sb = wp_.tile([128, 2 * CT], F32, tag="consts", name="consts")
            magic_sb = wp_.tile([128, 1], I32, tag="magic", name="magic")
            nc.vector.memset(magic_sb, MAGIC)
            expb_sb = wp_.tile([128, 1], F32, tag="expb", name="expb")
            nc.vector.memset(expb_sb, EXPB)
            nc.vector.memset(ones_sb.bitcast(F32), 1.0)
            gnsc_sb = consts_sb[:, 0 * CT:1 * CT]
            gnbi_sb = consts_sb[:, 1 * CT:2 * CT]
            if not zero_qk_bias:
                wc_all = wp_.tile([128, CT, 16], FP8, tag="wc", name="wc")

            def part(dram2d):
                # (T*128, F) -> [128, T, F]
                return dram2d.rearrange("(t p) f -> p t f", p=128)

            x0 = xpool.tile([128, CT, N], BF16, tag="x", name="x")
            nc.sync.dma_start(out=x0[:, 0, 0:512], in_=x_d[0, 0:128, 0:512])
            nc.sync.dma_start(out=ind_all, in_=part(ind_d[:, :]))
            nc.sync.dma_start(out=x0[:, 0, 512:1024], in_=x_d[0, 0:128, 512:1024])
            for t in range(1, CT):
                nc.sync.dma_start(
                    out=x0[:, t, :], in_=x_d[0, 128 * t:128 * (t + 1), :]
                )
            nc.sync.dma_start(out=consts_sb, in_=consts_d[:, :])
            nc.sync.dma_start(
                out=bind_all, in_=bind_d.rearrange("g (t p) -> g t p", p=128)
            )
            nc.sync.dma_start(out=mt_all, in_=part(mt_d[:, :]))
            nc.sync.dma_start(out=w2t_all, in_=part(w2t_d[:, :]))
            if not zero_qk_bias:
                nc.sync.dma_start(out=wc_all, in_=part(wc_d[:, :]))

            halves = [slice(0, 512), slice(512, 1024)]

            def emit_gn(img, x_pre=None):
                """Load x, GroupNorm stats + chain, xn apply (fp8 out)."""
                if x_pre is not None:
                    x_all = x_pre
                else:
                    x_all = xpool.tile([128, CT, N], BF16, tag="x", name="x")
                    for t in range(CT):
                        nc.sync.dma_start(
                            out=x_all[:, t, :],
                            in_=x_d[img, 128 * t:128 * (t + 1), :],
                        )
                xt = [x_all[:, t, :] for t in range(CT)]

                # per-channel stats on DVE: bn_stats windows + bn_aggr ->
                # chstat[:, t, :] = [mean_c, var_c, mean_c^2, 0]
                bnst = spool.tile([128, CT, 2, 6], F32, tag="bnst", name="bnst")
                chstat = spool.tile([128, CT, 4], F32, tag="chst", name="chst")
                recipbc = rpool.tile([128, N], F32, tag="rbc", name="rbc")
                nc.vector.memset(chstat[:, :, 3:4], 0.0)
                for t in range(CT):
                    for h in range(NH):
                        nc.vector.bn_stats(bnst[:, t, h, :], xt[t][:, halves[h]])
                    nc.vector.bn_aggr(chstat[:, t, 0:2], bnst[:, t, :, :])
                nc.vector.tensor_mul(chstat[:, :, 2:3], chstat[:, :, 0:1],
                                     chstat[:, :, 0:1])
                chr_ = spool.tile([128, CT, 4], F32R, tag="chr", name="chr")
                nc.vector.tensor_copy(chr_[:, :, :], chstat[:, :, :])
                # combine over the 16 channels/group: indicator (1/16) matmul
                # -> gstat = [mu_g, E[var_c], E[mean_c^2], 0]
                gstat = psB.tile([G, 4], F32, tag="psB", name="psB")
                for t in range(CT):
                    nc.tensor.matmul(
                        gstat[:, :], ind_all[:, t, :], chr_[:, t, :],
                        start=(t == 0), stop=(t == CT - 1),
                    )
                gsb = spool.tile([G, 3], F32, tag="gsb", name="gsb")
                nc.vector.tensor_copy(gsb[:, :], gstat[:, 0:3])
                # vpe = (E[var] + E[mean^2] + eps) - mu^2
                vpe = spool.tile([G, 1], F32, tag="vpe", name="vpe")
                msq = spool.tile([G, 1], F32, tag="msq", name="msq")
                e2p = spool.tile([G, 1], F32, tag="e2p", name="e2p")
                nc.vector.tensor_add(e2p[:, :], gsb[:, 1:2], gsb[:, 2:3])
                nc.vector.tensor_mul(msq[:, :], gsb[:, 0:1], gsb[:, 0:1])
                nc.vector.scalar_tensor_tensor(
                    out=vpe[:, :], in0=e2p[:, :], scalar=EPS, in1=msq[:, :],
                    op0=ALU.add, op1=ALU.subtract,
                )
                # rstd = 1/sqrt(vpe): bit-trick seed + 2 Newton iterations
                sh_t = spool.tile([G, 1], I32, tag="sh", name="sh")
                nc.vector.tensor_scalar(
                    out=sh_t[:, :], in0=vpe.bitcast(I32)[:, :], scalar1=1,
                    scalar2=None, op0=ALU.logical_shift_right,
                )
                seed = spool.tile([G, 1], I32, tag="seed", name="seed")
                nc.vector.scalar_tensor_tensor(
                    out=seed[:, :], in0=magic_sb[:G, :], scalar=0, in1=sh_t[:, :],
                    op0=ALU.bypass, op1=ALU.subtract,
                )
                y = seed.bitcast(F32)
                for it in range(2):
                    t1 = spool.tile([G, 1], F32, tag=f"nr{it}", name=f"nr{it}")
                    nc.vector.tensor_mul(t1[:, :], y[:, :], y[:, :])
                    nc.vector.tensor_mul(t1[:, :], t1[:, :], vpe[:, :])
                    nc.vector.tensor_scalar(
                        out=t1[:, :], in0=t1[:, :], scalar1=-0.5, scalar2=1.5,
                        op0=ALU.mult, op1=ALU.add,
                    )
                    y2 = spool.tile([G, 1], F32, tag=f"y{it}", name=f"y{it}")
                    nc.vector.tensor_mul(y2[:, :], y[:, :], t1[:, :])
                    y = y2
                # stats2 = [rstd, mean] (f32r for the broadcast matmul)
                stats2 = spool.tile([G, 2], F32R, tag="st2", name="st2")
                nc.vector.tensor_copy(stats2[:, 0:1], y[:, :])
                nc.vector.tensor_copy(stats2[:, 1:2], gsb[:, 0:1])

                # broadcast to channels; a = gnsc*rstd, b = gnbi - mean*a
                bc_ps = psB.tile([128, 2 * CT], F32, tag="psB", name="psB")
                for t in range(CT):
                    nc.tensor.matmul(
                        bc_ps[:, 2 * t:2 * t + 2], bind_all[:, t, :], stats2[:, :],
                        start=True, stop=True,
                    )
                bc_sb = spool.tile([128, 2 * CT], F32, tag="bc", name="bc")
                nc.vector.tensor_copy(bc_sb[:, :], bc_ps[:, :])
                bc_v = bc_sb.rearrange("p (t s) -> p t s", s=2)
                a_all = spool.tile([128, CT], F32, tag="aall", name="aall")
                b_all = spool.tile([128, CT], F32, tag="ball", name="ball")
                nc.vector.tensor_mul(a_all[:, :], gnsc_sb, bc_v[:, :, 0])
                nc.vector.scalar_tensor_tensor(
                    out=b_all[:, :], in0=bc_v[:, :, 1], scalar=-1.0, in1=a_all[:, :],
                    op0=ALU.mult, op1=ALU.mult,
                )
                nc.vector.tensor_add(b_all[:, :], b_all[:, :], gnbi_sb)

                # xn = a*x + b -> fp8 (ACT; Identity is in every table set)
                xn = xnpool.tile([128, CT, N], FP8, tag="xn", name="xn")
                for t in range(CT):
                    nc.scalar.activation(
                        out=xn[:, t, :], in_=xt[t], func=AF.Identity,
                        scale=a_all[:, t:t + 1], bias=b_all[:, t:t + 1],
                    )
                return {"xt": xt, "xn": xn, "recipbc": recipbc, "img": img}

            def emit_tv(s):
                """t = St*(M@xn) and v'T = (W2@xn)^T, both fp8 DoubleRow."""
                xn = s["xn"]
                tsb = qpool.tile([128, CT, N], FP8, tag="t", name="t")
                for d in range(CT):
                    ds_ = slice(128 * d, 128 * (d + 1))
                    tps = psA.tile([128, N], F32, tag="psA", name="psA")
                    for u in range(CT // 2):
                        for h in range(NH):
                            nc.tensor.matmul(
                                tps[:, halves[h]], mt_all[:, 2 * u:2 * u + 2, ds_],
                                xn[:, 2 * u:2 * u + 2, halves[h]],
                                start=(u == 0), stop=(u == CT // 2 - 1),
                                perf_mode=DR,
                            )
                    nc.scalar.activation(out=tsb[:, d, :], in_=tps[:, :],
                                         func=AF.Copy, scale=TS)
                # v'T[n, o]: lhsT = xn pair-slice, rhs = w2t pair-slice
                vT = vpool.tile([128, NT, C], FP8, tag="v", name="v")
                for n in range(NT):
                    ns = slice(128 * n, 128 * (n + 1))
                    vps = psB.tile([128, 512], F32, tag="psB", name="psB")
                    for u in range(CT // 2):
                        nc.tensor.matmul(
                            vps[:, :], xn[:, 2 * u:2 * u + 2, ns],
                            w2t_all[:, 2 * u:2 * u + 2, :],
                            start=(u == 0), stop=(u == CT // 2 - 1),
                            perf_mode=DR,
                        )
                    nc.scalar.activation(out=vT[:, n, :], in_=vps[:, :],
                                         func=AF.Copy, scale=VS)
                s["t"], s["vT"] = tsb, vT

            def emit_att(s):
                """scores gram, softmax, attn@v', residual, DMA out."""
                tsb, vT, xn, recipbc = s["t"], s["vT"], s["xn"], s["recipbc"]
                xt, img = s["xt"], s["img"]
                if not zero_qk_bias:
                    # c_j = scale*bq^T Wk xn_j, added to scores via exp bias
                    cbias = spool.tile([128, NT], F32, tag="cb", name="cb")
                    for n in range(NT):
                        ns = slice(128 * n, 128 * (n + 1))
                        cps = psB.tile([128, 16], F32, tag="psB", name="psB")
                        for u in range(CT // 2):
                            nc.tensor.matmul(
                                cps[:, :], xn[:, 2 * u:2 * u + 2, ns],
                                wc_all[:, 2 * u:2 * u + 2, :],
                                start=(u == 0), stop=(u == CT // 2 - 1),
                                perf_mode=DR,
                            )
                        nc.vector.tensor_scalar(
                            out=cbias[:, n:n + 1], in0=cps[:, 0:1],
                            scalar1=1.0 / SWC, scalar2=EXPB,
                            op0=ALU.mult, op1=ALU.add,
                        )
                # l[i] = sum_j exp: pair-adds interleaved with the exp loop
                # (they run on DVE under the scores matmuls), then 2+1 adds
                # and a small f32r ones-matmul for partition-reduce+broadcast
                expT = epool.tile([128, NT, N], FP8, tag="e", name="e")
                esum4 = espool.tile([128, 4, N], F32, tag="es4", name="es4")
                for j in range(NT):
                    js = slice(128 * j, 128 * (j + 1))
                    sps = psA.tile([128, N], F32, tag="psA", name="psA")
                    for u in range(CT // 2):
                        for h in range(NH):
                            nc.tensor.matmul(
                                sps[:, halves[h]], tsb[:, 2 * u:2 * u + 2, js],
                                xn[:, 2 * u:2 * u + 2, halves[h]],
                                start=(u == 0), stop=(u == CT // 2 - 1),
                                perf_mode=DR,
                            )
                    bias_ap = (expb_sb[:, :] if zero_qk_bias
                               else cbias[:, j:j + 1])
                    nc.scalar.activation(out=expT[:, j, :], in_=sps[:, :],
                                         func=AF.Exp, scale=1.0 / ST,
                                         bias=bias_ap)
                    if j % 2 == 1:
                        nc.vector.tensor_add(esum4[:, j // 2, :],
                                             expT[:, j - 1, :], expT[:, j, :])
                esum2 = espool.tile([128, 2, N], F32, tag="es2", name="es2")
                esum1 = espool.tile([128, N], F32R, tag="es1", name="es1")
                nc.vector.tensor_add(esum2[:, :, :], esum4[:, 0:2, :],
                                     esum4[:, 2:4, :])
                nc.vector.tensor_add(esum1[:, :], esum2[:, 0, :],
                                     esum2[:, 1, :])
                lps = psA.tile([128, N], F32, tag="psA", name="psA")
                for h in range(NH):
                    nc.tensor.matmul(
                        lps[:, halves[h]], ones_sb[:, :], esum1[:, halves[h]],
                        start=True, stop=True,
                    )
                nc.vector.reciprocal_approx_fast(out=recipbc[:, :], in_=lps[:, :])

                # attn@v' (contract over j); normalize + residual + DMA out
                fin_all = fpool.tile([128, CT, N], BF16, tag="f", name="f")
                for o in range(CT):
                    os_ = slice(128 * o, 128 * (o + 1))
                    aps = psA.tile([128, N], F32, tag="psA", name="psA")
                    for u in range(NT // 2):
                        for h in range(NH):
                            nc.tensor.matmul(
                                aps[:, halves[h]], vT[:, 2 * u:2 * u + 2, os_],
                                expT[:, 2 * u:2 * u + 2, halves[h]],
                                start=(u == 0), stop=(u == NT // 2 - 1),
                                perf_mode=DR,
                            )
                    tmp = tpool.tile([128, N], F32, tag="tmp", name="tmp")
                    nc.vector.tensor_mul(tmp[:, :], aps[:, :], recipbc[:, :])
                    for h in range(NH):
                        nc.vector.tensor_add(
                            fin_all[:, o, halves[h]], tmp[:, halves[h]],
                            xt[o][:, halves[h]],
                        )
                        nc.sync.dma_start(
                            out=out_d[img, 128 * o:128 * (o + 1), halves[h]],
                            in_=fin_all[:, o, halves[h]],
                        )

            def _body():
                seq = [i % BPC for i in range(BPC * repeat)]
                states = [None] * len(seq)
                states[0] = emit_gn(seq[0], x_pre=x0 if seq[0] == 0 else None)
                emit_tv(states[0])
                for i, img in enumerate(seq):
                    if i + 1 < len(seq):
                        states[i + 1] = emit_gn(seq[i + 1])
                    emit_att(states[i])
                    states[i] = None
                    if i + 1 < len(seq):
                        emit_tv(states[i + 1])

            if loop_iters:
                with tc.For_i(0, loop_iters, 1,
                              hint_engines=(mybir.EngineType.PE,
                                            mybir.EngineType.Activation,
                                            mybir.EngineType.DVE,
                                            mybir.EngineType.SP)):
                    _body()
            else:
                _body()

    nc.compile()
    return nc


def _prep_inputs(x, gn_scale, gn_bias, qkv_w, qkv_b, proj_w, proj_b,
                 attn_dt="fp8"):
    f = np.float32
    x_r = np.asarray(x, dtype=f).reshape(B, C, N)
    qkv_w = np.asarray(qkv_w, dtype=f)
    qkv_b = np.asarray(qkv_b, dtype=f)
    proj_w = np.asarray(proj_w, dtype=f)
    proj_b = np.asarray(proj_b, dtype=f)
    Wq, Wk, Wv = qkv_w[0:C], qkv_w[C:2 * C], qkv_w[2 * C:3 * C]
    # v-bias and proj-bias fold into a constant per-channel offset added to x
    # (rows of attn sum to 1): out += Wp @ bv + bp.
    bv = qkv_b[2 * C:3 * C]
    cvec = proj_w @ bv + proj_b
    if np.any(cvec):
        x_r = x_r + cvec[None, :, None]

    # merged weights (f64 composition, then fp8)
    M = (SCALE * (Wq.T.astype(np.float64) @ Wk.astype(np.float64))).astype(f)
    W2 = (proj_w.astype(np.float64) @ Wv.astype(np.float64)).astype(f)

    def w8(m, s):
        return np.clip(np.ascontiguousarray(m.T) * s, -240, 240).astype(FP8NP)

    def col(v):
        return np.asarray(v, f).reshape(CT, 128).T

    consts = np.concatenate([col(gn_scale), col(gn_bias)], axis=1)
    indicator = (np.arange(C)[:, None] // GS == np.arange(G)[None, :]).astype(f)
    common = {
        "mt": w8(M, SM),
        "w2t": w8(W2, SW2),
        "ind": np.ascontiguousarray(indicator / GS),
        "bind": np.ascontiguousarray(indicator.T),
        "consts": np.ascontiguousarray(consts),
    }
    bq, bk = qkv_b[0:C], qkv_b[C:2 * C]
    zero_qk = not (np.any(bq) or np.any(bk))
    if not zero_qk:
        # c_j = scale*bq^T Wk xn_j enters as a per-j exp bias; bk's row term
        # and the bq^T bk constant cancel in softmax.
        wc = np.zeros((C, 16), dtype=f)
        wc[:, 0] = SCALE * (Wk.T @ bq)
        common["wc"] = np.clip(wc * SWC, -240, 240).astype(FP8NP)
    x_bf = x_r.astype(ml_dtypes.bfloat16)
    in_maps = []
    for i in range(NCORES):
        m = dict(common)
        m["x"] = np.ascontiguousarray(x_bf[BPC * i:BPC * (i + 1)])
        in_maps.append(m)
    return in_maps, zero_qk


def kernel(x, gn_scale, gn_bias, qkv_w, qkv_b, proj_w, proj_b, _trace=False):
    in_maps, zero_qk = _prep_inputs(x, gn_scale, gn_bias, qkv_w, qkv_b,
                                    proj_w, proj_b)
    key = ("nc", zero_qk)
    if key not in _cache:
        _cache[key] = _build(zero_qk_bias=zero_qk)
    nc = _cache[key]
    res = run_bass_kernel_spmd(nc, in_maps, core_ids=list(range(NCORES)),
                               trace=_trace)
    _cache["last_result"] = res
    out = np.stack([np.asarray(r["out"]).astype(np.float32)
                    for r in res.results], axis=0)
    return out.reshape(B, C, H, W)


# revision 34
# speedup vs baseline: 1.1839x; 1.1839x over previous
"""AttentionBlock (GroupNorm(32) + 1-head self-attention + proj + residual) on 8 trn2 cores.

Data-parallel over batch: each of the 8 NeuronCores processes 2 of the 16 images.
All large matmuls run in fp8-e4m3 with perf_mode=DoubleRow (2 MACs/cell/cycle:
each matmul contracts K=256 via paired k-tiles in a 3D [128, 2, free] AP).

Weight merging (NUM_HEADS=1 makes adjacent linear maps composable host-side):
  - scores = xn^T (scale*Wq^T Wk) xn: precompute M, then t = M@xn (1 unit)
    + gram t(x)xn (2 units) replaces q,k,scores (4 units).
  - out = attn @ ((Wp Wv) xn): precompute W2, then v' = W2@xn (1) + attn@v'
    (2) replaces v,attn@v,proj (4). The attention output needs NO fp8
    roundtrip: PSUM -> (1/l mul) -> (+x residual) -> bf16 DMA directly.
  Net: 96 DoubleRow matmuls/image instead of 128 (25% PE cut), and fewer fp8
  quantization stages (better accuracy than the unmerged fp8 kernel).

Quantization scheme (tolerance is 2e-2; this lands ~1e-2):
  - M quantized at SM=512 (sigma_M ~ 0.002), W2 at SW2=32.
  - t stored as St*(M@xn), St=16; scores psum = St*score and the exp
    activation applies scale=1/St, bias=-1.5. The -1.5 shift keeps
    exp <= ~60 (TRN e4m3 overflows to Inf above 240) and cancels in softmax.
  - GroupNorm stats off the PE: per-channel mean/var via DVE bn_stats/bn_aggr,
    combined across the 16 channels/group by a tiny f32r indicator matmul.
  - Softmax denominator off the PE: a 3-add DVE tree collapses the 8 expT
    tiles, then one small f32r ones-matmul reduces partitions + broadcasts.

Layout strategy (per image, c=512 channels, n=1024 positions):
  - x bf16 [128, CT, N] (bf16 in/out DMA halves HBM traffic); xn/t fp8
    [128, CT, N] (channel, position); v'T fp8 [128, NT, C] (position,
    channel); expT fp8 [128, NT, N]. Pair-tiles adjacent in the free dim
    enable the [:, 2u:2u+2, ...] DoubleRow slices for both operands.
  - softmax over j: no max subtraction (scores ~N(0,1) and the -1.5 exp
    shift bounds the range).
  - rstd via Newton rsqrt on DVE; 1/l via reciprocal_approx_fast; ScalarE only
    needs one ACT table set (exp/copy/identity).
  - DMAs batched; per-phase emission order software-pipelines the two images.
"""

import ml_dtypes
import numpy as np

import concourse.bacc as bacc
import concourse.tile as tile
import concourse.mybir as mybir
from concourse.bass_utils import run_bass_kernel_spmd

F32 = mybir.dt.float32
F32R = mybir.dt.float32r
BF16 = mybir.dt.bfloat16
FP8 = mybir.dt.float8e4
I32 = mybir.dt.int32
AF = mybir.ActivationFunctionType
ALU = mybir.AluOpType
AX = mybir.AxisListType
DR = mybir.MatmulPerfMode.DoubleRow
FP8NP = ml_dtypes.float8_e4m3

B, C, H, W = 16, 512, 32, 32
N = H * W                 # 1024 positions
NCORES = 8
BPC = B // NCORES         # 2 images per core
G = 32                    # groupnorm groups
GS = C // G               # 16 channels per group
CT = C // 128             # 4 channel tiles
NT = N // 128             # 8 position tiles
NH = N // 512             # 2 free-dim halves
EPS = 1e-5
SCALE = float(C) ** -0.5  # single head, head_dim = C
MAGIC = 0x5F3759DF        # Newton-rsqrt seed constant

SM = 512.0                # M = scale*Wq^T@Wk quantization scale
SW2 = 32.0                # W2 = Wp@Wv quantization scale
ST = 16.0                 # t = M@xn storage prescale
SWC = 32.0                # bias-correction vector scale (nonzero-bias path)
EXPB = -1.5               # exp shift (cancels in softmax; keeps exp < 240)
TS = ST / SM              # t copy-out scale
VS = 1.0 / SW2            # v' copy-out scale

_cache: dict = {}

ATTN_DT = "fp8"


def _build(repeat: int = 1, zero_qk_bias: bool = True, loop_iters: int = 0,
           attn_dtype=None):
    nc = bacc.Bacc("TRN2", target_bir_lowering=False, num_devices=NCORES)

    x_d = nc.dram_tensor("x", [BPC, C, N], BF16, kind="ExternalInput")
    mt_d = nc.dram_tensor("mt", [C, C], FP8, kind="ExternalInput")     # M.T * SM
    w2t_d = nc.dram_tensor("w2t", [C, C], FP8, kind="ExternalInput")   # W2.T * SW2
    ind_d = nc.dram_tensor("ind", [C, G], F32R, kind="ExternalInput")  # 1/16 iff c//16==g
    bind_d = nc.dram_tensor("bind", [G, C], F32R, kind="ExternalInput")  # 0/1 indicator.T
    # consts: [gnsc | gnbi], each (128, CT)
    consts_d = nc.dram_tensor("consts", [128, 2 * CT], F32, kind="ExternalInput")
    if not zero_qk_bias:
        wc_d = nc.dram_tensor("wc", [C, 16], FP8, kind="ExternalInput")  # SWC*scale*Wk^T@bq (col 0)
    out_d = nc.dram_tensor("out", [BPC, C, N], BF16, kind="ExternalOutput")

    with tile.TileContext(nc) as tc:
        with (
            tc.tile_pool(name="wpool", bufs=1) as wp_,
            tc.tile_pool(name="xpool", bufs=2) as xpool,
            tc.tile_pool(name="xnpool", bufs=2) as xnpool,
            tc.tile_pool(name="espool", bufs=1) as espool,
            tc.tile_pool(name="qpool", bufs=2) as qpool,
            tc.tile_pool(name="vpool", bufs=2) as vpool,
            tc.tile_pool(name="epool", bufs=2) as epool,
            tc.tile_pool(name="fpool", bufs=1) as fpool,
            tc.tile_pool(name="tpool", bufs=2) as tpool,
            tc.tile_pool(name="rpool", bufs=2) as rpool,
            tc.tile_pool(name="spool", bufs=2) as spool,
            tc.tile_pool(name="psA", bufs=3, space="PSUM") as psA,
            tc.tile_pool(name="psB", bufs=2, space="PSUM") as psB,
        ):
            # ---- persistent constants / weights (batched single DMAs) ----
            mt_all = wp_.tile([128, CT, C], FP8, tag="mt", name="mt")
            w2t_all = wp_.tile([128, CT, C], FP8, tag="w2t", name="w2t")
            ind_all = wp_.tile([128, CT, G], F32R, tag="ind", name="ind")
            bind_all = wp_.tile([G, CT, 128], F32R, tag="bind", name="bind")
            ones_sb = wp_.tile([128, 128], F32R, tag="ones", name="ones")
            consts_sb = wp_.tile([128, 2 * CT], F32, tag="consts", name="consts")
            magic_sb = wp_.tile([128, 1], I32, tag="magic", name="magic")
            nc.vector.memset(magic_sb, MAGIC)
            expb_sb = wp_.tile([128, 1], F32, tag="expb", name="expb")
            nc.vector.memset(expb_sb, EXPB)
            nc.vector.memset(ones_sb.bitcast(F32), 1.0)
            gnsc_sb = consts_sb[:, 0 * CT:1 * CT]
            gnbi_sb = consts_sb[:, 1 * CT:2 * CT]
            if not zero_qk_bias:
                wc_all = wp_.tile([128, CT, 16], FP8, tag="wc", name="wc")

            def part(dram2d):
                # (T*128, F) -> [128, T, F]
                return dram2d.rearrange("(t p) f -> p t f", p=128)

            x0 = xpool.tile([128, CT, N], BF16, tag="x", name="x")
            nc.sync.dma_start(out=x0[:, 0, 0:512], in_=x_d[0, 0:128, 0:512])
            nc.sync.dma_start(out=ind_all, in_=part(ind_d[:, :]))
            nc.sync.dma_start(out=x0[:, 0, 512:1024], in_=x_d[0, 0:128, 512:1024])
            for t in range(1, CT):
                nc.sync.dma_start(
                    out=x0[:, t, :], in_=x_d[0, 128 * t:128 * (t + 1), :]
                )
            nc.sync.dma_start(out=consts_sb, in_=consts_d[:, :])
            nc.sync.dma_start(
                out=bind_all, in_=bind_d.rearrange("g (t p) -> g t p", p=128)
            )
            nc.sync.dma_start(out=mt_all, in_=part(mt_d[:, :]))
            nc.sync.dma_start(out=w2t_all, in_=part(w2t_d[:, :]))
            if not zero_qk_bias:
                nc.sync.dma_start(out=wc_all, in_=part(wc_d[:, :]))

            halves = [slice(0, 512), slice(512, 1024)]

            def emit_gn(img, x_pre=None):
                """Load x, GroupNorm stats + chain, xn apply (fp8 out)."""
                if x_pre is not None:
                    x_all = x_pre
                else:
                    x_all = xpool.tile([128, CT, N], BF16, tag="x", name="x")
                    for t in range(CT):
                        nc.sync.dma_start(
                            out=x_all[:, t, :],
                            in_=x_d[img, 128 * t:128 * (t + 1), :],
                        )
                xt = [x_all[:, t, :] for t in range(CT)]

                # per-channel stats on DVE: bn_stats windows + bn_aggr ->
                # chstat[:, t, :] = [mean_c, var_c, mean_c^2, 0]
                bnst = spool.tile([128, CT, 2, 6], F32, tag="bnst", name="bnst")
                chstat = spool.tile([128, CT, 4], F32, tag="chst", name="chst")
                recipbc = rpool.tile([128, N], F32, tag="rbc", name="rbc")
                nc.vector.memset(chstat[:, :, 3:4], 0.0)
                for t in range(CT):
                    for h in range(NH):
                        nc.vector.bn_stats(bnst[:, t, h, :], xt[t][:, halves[h]])
                    nc.vector.bn_aggr(chstat[:, t, 0:2], bnst[:, t, :, :])
                nc.vector.tensor_mul(chstat[:, :, 2:3], chstat[:, :, 0:1],
                                     chstat[:, :, 0:1])
                chr_ = spool.tile([128, CT, 4], F32R, tag="chr", name="chr")
                nc.vector.tensor_copy(chr_[:, :, :], chstat[:, :, :])
                # combine over the 16 channels/group: indicator (1/16) matmul
                # -> gstat = [mu_g, E[var_c], E[mean_c^2], 0]
                gstat = psB.tile([G, 4], F32, tag="psB", name="psB")
                for t in range(CT):
                    nc.tensor.matmul(
                        gstat[:, :], ind_all[:, t, :], chr_[:, t, :],
                        start=(t == 0), stop=(t == CT - 1),
                    )
                gsb = spool.tile([G, 3], F32, tag="gsb", name="gsb")
                nc.vector.tensor_copy(gsb[:, :], gstat[:, 0:3])
                # vpe = (E[var] + E[mean^2] + eps) - mu^2
                vpe = spool.tile([G, 1], F32, tag="vpe", name="vpe")
                msq = spool.tile([G, 1], F32, tag="msq", name="msq")
                e2p = spool.tile([G, 1], F32, tag="e2p", name="e2p")
                nc.vector.tensor_add(e2p[:, :], gsb[:, 1:2], gsb[:, 2:3])
                nc.vector.tensor_mul(msq[:, :], gsb[:, 0:1], gsb[:, 0:1])
                nc.vector.scalar_tensor_tensor(
                    out=vpe[:, :], in0=e2p[:, :], scalar=EPS, in1=msq[:, :],
                    op0=ALU.add, op1=ALU.subtract,
                )
                # rstd = 1/sqrt(vpe): bit-trick seed + 2 Newton iterations
                sh_t = spool.tile([G, 1], I32, tag="sh", name="sh")
                nc.vector.tensor_scalar(
                    out=sh_t[:, :], in0=vpe.bitcast(I32)[:, :], scalar1=1,
                    scalar2=None, op0=ALU.logical_shift_right,
                )
                seed = spool.tile([G, 1], I32, tag="seed", name="seed")
                nc.vector.scalar_tensor_tensor(
                    out=seed[:, :], in0=magic_sb[:G, :], scalar=0, in1=sh_t[:, :],
                    op0=ALU.bypass, op1=ALU.subtract,
                )
                y = seed.bitcast(F32)
                for it in range(2):
                    t1 = spool.tile([G, 1], F32, tag=f"nr{it}", name=f"nr{it}")
                    nc.vector.tensor_mul(t1[:, :], y[:, :], y[:, :])
                    nc.vector.tensor_mul(t1[:, :], t1[:, :], vpe[:, :])
                    nc.vector.tensor_scalar(
                        out=t1[:, :], in0=t1[:, :], scalar1=-0.5, scalar2=1.5,
                        op0=ALU.mult, op1=ALU.add,
                    )
                    y2 = spool.tile([G, 1], F32, tag=f"y{it}", name=f"y{it}")
                    nc.vector.tensor_mul(y2[:, :], y[:, :], t1[:, :])
                    y = y2
                # stats2 = [rstd, mean] (f32r for the broadcast matmul)
                stats2 = spool.tile([G, 2], F32R, tag="st2", name="st2")
                nc.vector.tensor_copy(stats2[:, 0:1], y[:, :])
                nc.vector.tensor_copy(stats2[:, 1:2], gsb[:, 0:1])

                # broadcast to channels; a = gnsc*rstd, b = gnbi - mean*a
                bc_ps = psB.tile([128, 2 * CT], F32, tag="psB", name="psB")
                for t in range(CT):
                    nc.tensor.matmul(
                        bc_ps[:, 2 * t:2 * t + 2], bind_all[:, t, :], stats2[:, :],
                        start=True, stop=True,
                    )
                bc_sb = spool.tile([128, 2 * CT], F32, tag="bc", name="bc")
                nc.vector.tensor_copy(bc_sb[:, :], bc_ps[:, :])
                bc_v = bc_sb.rearrange("p (t s) -> p t s", s=2)
                a_all = spool.tile([128, CT], F32, tag="aall", name="aall")
                b_all = spool.tile([128, CT], F32, tag="ball", name="ball")
                nc.vector.tensor_mul(a_all[:, :], gnsc_sb, bc_v[:, :, 0])
                nc.vector.scalar_tensor_tensor(
                    out=b_all[:, :], in0=bc_v[:, :, 1], scalar=-1.0, in1=a_all[:, :],
                    op0=ALU.mult, op1=ALU.mult,
                )
                nc.vector.tensor_add(b_all[:, :], b_all[:, :], gnbi_sb)

                # xn = a*x + b -> fp8 (ACT; Identity is in every table set)
                xn = xnpool.tile([128, CT, N], FP8, tag="xn", name="xn")
                for t in range(CT):
                    nc.scalar.activation(
                        out=xn[:, t, :], in_=xt[t], func=AF.Identity,
                        scale=a_all[:, t:t + 1], bias=b_all[:, t:t + 1],
                    )
                return {"xt": xt, "xn": xn, "recipbc": recipbc, "img": img}

            def emit_tv(s):
                """t = St*(M@xn) and v'T = (W2@xn)^T, both fp8 DoubleRow."""
                xn = s["xn"]
                tsb = qpool.tile([128, CT, N], FP8, tag="t", name="t")
                for d in range(CT):
                    ds_ = slice(128 * d, 128 * (d + 1))
                    tps = psA.tile([128, N], F32, tag="psA", name="psA")
                    for u in range(CT // 2):
                        for h in range(NH):
                            nc.tensor.matmul(
                                tps[:, halves[h]], mt_all[:, 2 * u:2 * u + 2, ds_],
                                xn[:, 2 * u:2 * u + 2, halves[h]],
                                start=(u == 0), stop=(u == CT // 2 - 1),
                                perf_mode=DR,
                            )
                    nc.scalar.activation(out=tsb[:, d, :], in_=tps[:, :],
                                         func=AF.Copy, scale=TS)
                # v'T[n, o]: lhsT = xn pair-slice, rhs = w2t pair-slice
                vT = vpool.tile([128, NT, C], FP8, tag="v", name="v")
                for n in range(NT):
                    ns = slice(128 * n, 128 * (n + 1))
                    vps = psB.tile([128, 512], F32, tag="psB", name="psB")
                    for u in range(CT // 2):
                        nc.tensor.matmul(
                            vps[:, :], xn[:, 2 * u:2 * u + 2, ns],
                            w2t_all[:, 2 * u:2 * u + 2, :],
                            start=(u == 0), stop=(u == CT // 2 - 1),
                            perf_mode=DR,
                        )
                    nc.vector.tensor_scalar(
                        out=vT[:, n, :], in0=vps[:, :], scalar1=VS,
                        scalar2=None, op0=ALU.mult,
                    )
                s["t"], s["vT"] = tsb, vT

            def emit_att(s):
                """scores gram, softmax, attn@v', residual, DMA out."""
                tsb, vT, xn, recipbc = s["t"], s["vT"], s["xn"], s["recipbc"]
                xt, img = s["xt"], s["img"]
                if not zero_qk_bias:
                    # c_j = scale*bq^T Wk xn_j, added to scores via exp bias
                    cbias = spool.tile([128, NT], F32, tag="cb", name="cb")
                    for n in range(NT):
                        ns = slice(128 * n, 128 * (n + 1))
                        cps = psB.tile([128, 16], F32, tag="psB", name="psB")
                        for u in range(CT // 2):
                            nc.tensor.matmul(
                                cps[:, :], xn[:, 2 * u:2 * u + 2, ns],
                                wc_all[:, 2 * u:2 * u + 2, :],
                                start=(u == 0), stop=(u == CT // 2 - 1),
                                perf_mode=DR,
                            )
                        nc.vector.tensor_scalar(
                            out=cbias[:, n:n + 1], in0=cps[:, 0:1],
                            scalar1=1.0 / SWC, scalar2=EXPB,
                            op0=ALU.mult, op1=ALU.add,
                        )
                # l[i] = sum_j exp: pair-adds interleaved with the exp loop
                # (they run on DVE under the scores matmuls), then 2+1 adds
                # and a small f32r ones-matmul for partition-reduce+broadcast
                expT = epool.tile([128, NT, N], FP8, tag="e", name="e")
                esum4 = espool.tile([128, 4, N], F32, tag="es4", name="es4")
                for j in range(NT):
                    js = slice(128 * j, 128 * (j + 1))
                    sps = psA.tile([128, N], F32, tag="psA", name="psA")
                    for u in range(CT // 2):
                        for h in range(NH):
                            nc.tensor.matmul(
                                sps[:, halves[h]], tsb[:, 2 * u:2 * u + 2, js],
                                xn[:, 2 * u:2 * u + 2, halves[h]],
                                start=(u == 0), stop=(u == CT // 2 - 1),
                                perf_mode=DR,
                            )
                    bias_ap = (expb_sb[:, :] if zero_qk_bias
                               else cbias[:, j:j + 1])
                    nc.scalar.activation(out=expT[:, j, :], in_=sps[:, :],
                                         func=AF.Exp, scale=1.0 / ST,
                                         bias=bias_ap)
                    if j % 2 == 1:
                        nc.vector.tensor_add(esum4[:, j // 2, :],
                                             expT[:, j - 1, :], expT[:, j, :])
                esum2 = espool.tile([128, 2, N], F32, tag="es2", name="es2")
                esum1 = espool.tile([128, N], F32R, tag="es1", name="es1")
                nc.vector.tensor_add(esum2[:, :, :], esum4[:, 0:2, :],
                                     esum4[:, 2:4, :])
                nc.vector.tensor_add(esum1[:, :], esum2[:, 0, :],
                                     esum2[:, 1, :])
                lps = psA.tile([128, N], F32, tag="psA", name="psA")
                for h in range(NH):
                    nc.tensor.matmul(
                        lps[:, halves[h]], ones_sb[:, :], esum1[:, halves[h]],
                        start=True, stop=True,
                    )
                nc.vector.reciprocal_approx_fast(out=recipbc[:, :], in_=lps[:, :])

                # attn@v' (contract over j); normalize + residual + DMA out
                fin_all = fpool.tile([128, CT, N], BF16, tag="f", name="f")
                for o in range(CT):
                    os_ = slice(128 * o, 128 * (o + 1))
                    aps = psA.tile([128, N], F32, tag="psA", name="psA")
                    for u in range(NT // 2):
                        for h in range(NH):
                            nc.tensor.matmul(
                                aps[:, halves[h]], vT[:, 2 * u:2 * u + 2, os_],
                                expT[:, 2 * u:2 * u + 2, halves[h]],
                                start=(u == 0), stop=(u == NT // 2 - 1),
                                perf_mode=DR,
                            )
                    tmp = tpool.tile([128, N], F32, tag="tmp", name="tmp")
                    nc.vector.tensor_mul(tmp[:, :], aps[:, :], recipbc[:, :])
                    for h in range(NH):
                        nc.vector.tensor_add(
                            fin_all[:, o, halves[h]], tmp[:, halves[h]],
                            xt[o][:, halves[h]],
                        )
                        nc.sync.dma_start(
                            out=out_d[img, 128 * o:128 * (o + 1), halves[h]],
                            in_=fin_all[:, o, halves[h]],
                        )

            def _body():
                seq = [i % BPC for i in range(BPC * repeat)]
                states = [None] * len(seq)
                states[0] = emit_gn(seq[0], x_pre=x0 if seq[0] == 0 else None)
                emit_tv(states[0])
                for i, img in enumerate(seq):
                    if i + 1 < len(seq):
                        states[i + 1] = emit_gn(seq[i + 1])
                    emit_att(states[i])
                    states[i] = None
                    if i + 1 < len(seq):
                        emit_tv(states[i + 1])

            if loop_iters:
                with tc.For_i(0, loop_iters, 1,
                              hint_engines=(mybir.EngineType.PE,
                                            mybir.EngineType.Activation,
                                            mybir.EngineType.DVE,
                                            mybir.EngineType.SP)):
                    _body()
            else:
                _body()

    nc.compile()
    return nc


def _prep_inputs(x, gn_scale, gn_bias, qkv_w, qkv_b, proj_w, proj_b,
                 attn_dt="fp8"):
    f = np.float32
    x_r = np.asarray(x, dtype=f).reshape(B, C, N)
    qkv_w = np.asarray(qkv_w, dtype=f)
    qkv_b = np.asarray(qkv_b, dtype=f)
    proj_w = np.asarray(proj_w, dtype=f)
    proj_b = np.asarray(proj_b, dtype=f)
    Wq, Wk, Wv = qkv_w[0:C], qkv_w[C:2 * C], qkv_w[2 * C:3 * C]
    # v-bias and proj-bias fold into a constant per-channel offset added to x
    # (rows of attn sum to 1): out += Wp @ bv + bp.
    bv = qkv_b[2 * C:3 * C]
    cvec = proj_w @ bv + proj_b
    if np.any(cvec):
        x_r = x_r + cvec[None, :, None]

    # merged weights (f64 composition, then fp8)
    M = (SCALE * (Wq.T.astype(np.float64) @ Wk.astype(np.float64))).astype(f)
    W2 = (proj_w.astype(np.float64) @ Wv.astype(np.float64)).astype(f)

    def w8(m, s):
        return np.clip(np.ascontiguousarray(m.T) * s, -240, 240).astype(FP8NP)

    def col(v):
        return np.asarray(v, f).reshape(CT, 128).T

    consts = np.concatenate([col(gn_scale), col(gn_bias)], axis=1)
    indicator = (np.arange(C)[:, None] // GS == np.arange(G)[None, :]).astype(f)
    common = {
        "mt": w8(M, SM),
        "w2t": w8(W2, SW2),
        "ind": np.ascontiguousarray(indicator / GS),
        "bind": np.ascontiguousarray(indicator.T),
        "consts": np.ascontiguousarray(consts),
    }
    bq, bk = qkv_b[0:C], qkv_b[C:2 * C]
    zero_qk = not (np.any(bq) or np.any(bk))
    if not zero_qk:
        # c_j = scale*bq^T Wk xn_j enters as a per-j exp bias; bk's row term
        # and the bq^T bk constant cancel in softmax.
        wc = np.zeros((C, 16), dtype=f)
        wc[:, 0] = SCALE * (Wk.T @ bq)
        common["wc"] = np.clip(wc * SWC, -240, 240).astype(FP8NP)
    x_bf = x_r.astype(ml_dtypes.bfloat16)
    in_maps = []
    for i in range(NCORES):
        m = dict(common)
        m["x"] = np.ascontiguousarray(x_bf[BPC * i:BPC * (i + 1)])
        in_maps.append(m)
    return in_maps, zero_qk


def kernel(x, gn_scale, gn_bias, qkv_w, qkv_b, proj_w, proj_b, _trace=False):
    in_maps, zero_qk = _prep_inputs(x, gn_scale, gn_bias, qkv_w, qkv_b,
                                    proj_w, proj_b)
    key = ("nc", zero_qk)
    if key not in _cache:
        _cache[key] = _build(zero_qk_bias=zero_qk)
    nc = _cache[key]
    res = run_bass_kernel_spmd(nc, in_maps, core_ids=list(range(NCORES)),
                               trace=_trace)
    _cache["last_result"] = res
    out = np.stack([np.asarray(r["out"]).astype(np.float32)
                    for r in res.results], axis=0)
    return out.reshape(B, C, H, W)
